# revision 51
# baseline (speedup 1.0000x reference)
"""Trainium2 Bass kernel for nn_AttentionSampling (sparse window attention block).

Sharding: 8 cores, data-parallel, 1024 windows (half a batch) per core; windows are
independent so there is no cross-core communication. q/k live in a transposed
[d, tokens] layout (host pre-transposes) so projections run weight-stationary;
v stays in natural [keys, d] layout so the banded attention aggregation can run
as PE matmuls against the masked score matrix.

Structure (per 128-window / 512-key block):
- k-proj (N=512 bf16 matmuls) -> scores computed directly TRANSPOSED
  ([keys, windows], 16 N=128 matmuls) -> DVE band-mask multiply produces the
  sparse weight matrix W [512 keys, 128 windows] in bf16.
- Attention output via aggregate-then-project: avT = v_nat.T-contracted with W
  (16 N=128 MMs per block); the wv projection + residual add + LN1 stats then
  run once per 512-token superblock at N=512 (wv does not see the block
  structure). 4x fewer v-projection MACs than project-then-aggregate, no PE
  transposes.
- LayerNorm stats are bf16 ones-matmuls pipelined into the producer streams;
  mean/rstd broadcasts are rank-1 matmuls; the scalar chain is 5 hops; the
  apply is 2 DVE passes (ACT affine only when g != 1 or b != 0, decided at
  runtime from the actual inputs, as are the zero v/ffn biases).
- Software pipeline: iteration b emits k-proj(b), scoresT(b-1), v-agg(b-2);
  every PE group's ACT/DVE dependencies are >=1 block old.
- DMA issue follows global need order across the sync/scalar/gpsimd queues
  (aggregate ~330 GB/s is shared; engine boot blocks issue until ~7.6us);
  PE warmup matmuls bridge the initial fill so the HAM clock gate is open
  when real work starts.
- LN2 apply + output DMA chunked per d-tile to shrink the serial tail.
"""

import sys
import types

# If BASS_TRACE is set in an environment whose antenv package lacks
# axon_hooks, run_bass_kernel_spmd would crash on import; provide a stub
# (a None hook makes bass_utils skip tracing gracefully).
try:
    import antenv.axon_hooks  # noqa: F401
except ImportError:
    _m = types.ModuleType("antenv.axon_hooks")
    _m.get_axon_ntff_profile_hook = lambda: None
    _m.set_axon_ntff_profile_hook = lambda h: None
    sys.modules["antenv.axon_hooks"] = _m
    try:
        import antenv

        antenv.axon_hooks = _m
    except ImportError:
        pass

import contextlib

import numpy as np

import concourse.bass as bass
import concourse.bacc as bacc_mod
import concourse.mybir as mybir
import concourse.tile as tile
from concourse.bass import ts, ds
from concourse.bass_utils import run_bass_kernel_spmd

FP32 = mybir.dt.float32
FP16 = mybir.dt.float16
AF = mybir.ActivationFunctionType
OP = mybir.AluOpType

MM_DT = mybir.dt.bfloat16  # matmul operands
# The residual stream and LN stats run in bf16 (fp16 matmuls measured at HALF
# the bf16 rate on TRN2 HW despite the cost model saying otherwise; bf16
# element quantization only costs ~1e-2 worst-element relative error vs the
# 2e-2 gate). PSUM accumulation is fp32. rstd and the centered apply scratch
# stay fp16 (11-bit mantissa) since they multiply the output directly.

B, SQ, SK, D, F = 4, 2048, 8192, 512, 4
NCORES = 8
WPC = B * SQ // NCORES        # 1024 windows (= tokens) per core
KPC = WPC * F                 # 4096 keys per core
NBLK = WPC // 128             # 8 attention blocks: 128 windows / 512 keys
NSB = WPC // 512              # 2 superblocks of 512 tokens
DT = D // 128                 # 4 d-tiles
KC = 4                        # key chunks per block (512 keys / 128)
EPS = 1e-5
N_WARMUP = 10                 # PE warmup matmuls during initial DMA fill

_CACHE = {}


def build_program(use_vbias=True, affine1=True, affine2=True):
    nc = bacc_mod.Bacc(None, target_bir_lowering=False)

    qT_d = nc.dram_tensor("qT", [D, WPC], MM_DT, kind="ExternalInput")
    kT_d = nc.dram_tensor("kT", [D, KPC], MM_DT, kind="ExternalInput")
    vN_d = nc.dram_tensor("vN", [KPC, D], MM_DT, kind="ExternalInput")
    wq_d = nc.dram_tensor("w_q", [D, D], MM_DT, kind="ExternalInput")
    wk_d = nc.dram_tensor("w_k", [D, D], MM_DT, kind="ExternalInput")
    wv_d = nc.dram_tensor("w_v", [D, D], MM_DT, kind="ExternalInput")
    w1_d = nc.dram_tensor("ffn_w1", [D, D], MM_DT, kind="ExternalInput")
    w2_d = nc.dram_tensor("ffn_w2", [D, D], MM_DT, kind="ExternalInput")
    # All [D] bias/gain vectors are packed host-side into one [128, 8*DT]
    # tensor (order: b_q, b_k, ffn_b1, ffn_b2, ln1_g, ln1_b, ln2_g, ln2_b):
    # loading them individually as 4-byte-element gather DMAs costs 4-9us of
    # descriptor generation EACH and clogs the issuing engine's queue.
    consts_d = nc.dram_tensor("constsP", [128, 8 * DT], FP32, kind="ExternalInput")
    bvr_d = nc.dram_tensor("bv_row", [1, D], MM_DT, kind="ExternalInput")
    maskT_d = nc.dram_tensor("maskT", [128, KC, 128], FP32, kind="ExternalInput")
    outT_d = nc.dram_tensor("outT", [D, WPC], FP32, kind="ExternalOutput")

    qT_t = qT_d.rearrange("(o p) n -> p o n", p=128)
    kT_t = kT_d.rearrange("(o p) n -> p o n", p=128)
    vN_t = vN_d.rearrange("(nb kc p) d -> p nb kc d", p=128, kc=KC)
    outT_t = outT_d.rearrange("(o p) n -> p o n", p=128)

    with tile.TileContext(nc) as tc, contextlib.ExitStack() as ctx:
        # PSUM budget is 8 banks x 2KB: proj(2) + sc(1) + av(1) + ao(1|2) +
        # stats/bc shared tag(2) [+ srow(1) on the biased path] = 8.
        singles = ctx.enter_context(tc.tile_pool(name="singles", bufs=1))
        qin_p = ctx.enter_context(tc.tile_pool(name="qin", bufs=2))
        kin_p = ctx.enter_context(tc.tile_pool(name="kin", bufs=3))
        vin_p = ctx.enter_context(tc.tile_pool(name="vin", bufs=5))
        ktp_p = ctx.enter_context(tc.tile_pool(name="ktp", bufs=2))
        w_p = ctx.enter_context(tc.tile_pool(name="wsb", bufs=2))
        av_p = ctx.enter_context(tc.tile_pool(name="avsb", bufs=2))
        resid_p = ctx.enter_context(tc.tile_pool(name="resid", bufs=2))
        hT_p = ctx.enter_context(tc.tile_pool(name="hT", bufs=2))
        out_p = ctx.enter_context(tc.tile_pool(name="outp", bufs=2))
        small = ctx.enter_context(tc.tile_pool(name="small", bufs=1))
        ps_proj = ctx.enter_context(tc.tile_pool(name="ps_proj", bufs=2, space="PSUM"))
        ps_sc = ctx.enter_context(tc.tile_pool(name="ps_sc", bufs=1, space="PSUM"))
        ps_av = ctx.enter_context(tc.tile_pool(name="ps_av", bufs=1, space="PSUM"))
        ps_ao = ctx.enter_context(
            tc.tile_pool(name="ps_ao", bufs=1 if use_vbias else 2, space="PSUM"))
        ps_misc = ctx.enter_context(tc.tile_pool(name="ps_misc", bufs=2, space="PSUM"))

        # DMAs are spread across engine queues: single-queue DMA bandwidth
        # (~170-200 GB/s) is well below the core's aggregate, and the early
        # input fill is latency-critical for the PE.
        def load_w(d, tg, eng=None):
            t = singles.tile([128, DT, 512], MM_DT, tag=tg)
            (eng or nc.scalar).dma_start(out=t, in_=d.rearrange("(o p) n -> p o n", p=128))
            return t

        consts_sb = singles.tile([128, 8 * DT], FP32, tag="constsP")
        _CONST_IDX = {"b_q": 0, "b_k": 1, "ffn_b1": 2, "ffn_b2": 3,
                      "ln1_g": 4, "ln1_b": 5, "ln2_g": 6, "ln2_b": 7}

        def load_b(name):
            i = _CONST_IDX[name]
            return consts_sb[:, i * DT : (i + 1) * DT]

        # Warmup scratch needs no DMA: memset, then matmuls on it below.
        # Total DMA bandwidth (~330 GB/s) is shared across queues, so issue
        # order must be GLOBAL need order; multiple queues only parallelize
        # the ~0.7-1.9us per-issue instruction cost. Engine boot blocks all
        # DMA issue until ~7.6us, so the first-needed bytes are minimal:
        # wq+q0 (warmup covers them), then q1, wk+k0, k1; v/wv/ffn weights
        # stream in behind.
        warm_sb = singles.tile([128, 512], MM_DT, tag="warm")
        nc.gpsimd.memset(warm_sb, 0.001)
        ones_colb = singles.tile([128, 1], MM_DT, tag="ones_colb")
        nc.gpsimd.memset(ones_colb, 1.0)
        ones_rowb = singles.tile([1, 128], MM_DT, tag="ones_rowb")
        nc.gpsimd.memset(ones_rowb, 1.0)
        ones_rowh = singles.tile([1, 128], FP16, tag="ones_rowh")
        nc.gpsimd.memset(ones_rowh, 1.0)

        k_tiles = {}
        v_tiles = {}

        def prefetch_k(b):
            if b >= NBLK:
                return
            k_t = kin_p.tile([128, DT, 512], MM_DT, tag="k_in", name="k_in")
            nc.sync.dma_start(out=k_t, in_=kT_t[:, :, ts(b, 512)])
            k_tiles[b] = k_t

        def prefetch_v(b):
            if b >= NBLK:
                return
            v_t = vin_p.tile([128, KC, 512], MM_DT, tag="v_in", name="v_in")
            nc.gpsimd.dma_start(out=v_t, in_=vN_t[:, b, :, :])
            v_tiles[b] = v_t

        wq_sb = load_w(wq_d, "wq")
        nc.scalar.dma_start(out=consts_sb, in_=consts_d[:, :])
        q_in = []
        for sb in range(NSB):
            t = qin_p.tile([128, DT, 512], MM_DT, tag="q_in", name="q_in")
            q_in.append(t)
        nc.sync.dma_start(out=q_in[0], in_=qT_t[:, :, ts(0, 512)])
        wk_sb = load_w(wk_d, "wk")
        nc.sync.dma_start(out=q_in[1], in_=qT_t[:, :, ts(1, 512)])
        prefetch_k(0)
        bq_sb = load_b("b_q")
        bk_sb = load_b("b_k")
        prefetch_k(1)
        maskT = singles.tile([128, KC, 128], FP32, tag="maskT")
        nc.gpsimd.dma_start(out=maskT, in_=maskT_d[:, :, :])
        bv_row = None
        if use_vbias:
            bv_row = singles.tile([1, 512], MM_DT, tag="bv_row")
            nc.scalar.dma_start(out=bv_row, in_=bvr_d[:, :])

        late = {}

        def load_late_consts():
            late["wv"] = load_w(wv_d, "wv", eng=nc.gpsimd)
            late["g1"] = load_b("ln1_g")
            late["gb1"] = load_b("ln1_b")
            late["w1"] = load_w(w1_d, "w1", eng=nc.gpsimd)
            late["b1"] = load_b("ffn_b1")
            late["w2"] = load_w(w2_d, "w2", eng=nc.gpsimd)
            late["b2"] = load_b("ffn_b2")
            late["g2"] = load_b("ln2_g")
            late["gb2"] = load_b("ln2_b")

        # ---- PE warmup: trip the HAM clock gate while DMAs fill ----
        for i in range(N_WARMUP):
            wps = ps_proj.tile([128, 512], FP32, tag="proj_ps", name="warm_ps")
            nc.tensor.matmul(wps, lhsT=warm_sb[:, :128], rhs=warm_sb,
                             start=True, stop=True)

        qTp = singles.tile([128, DT, WPC], MM_DT, tag="qTp")
        xT = singles.tile([128, DT, WPC], MM_DT, tag="xT")

        def proj_T(w_sb, bias_sb, in_sb, out_sb, out_col0, n):
            for do in range(DT):
                ps = ps_proj.tile([128, 512], FP32, tag="proj_ps", name="proj_ps")
                ps = ps[:, :n]
                for ki in range(DT):
                    nc.tensor.matmul(
                        ps, lhsT=w_sb[:, ki, ts(do, 128)], rhs=in_sb[:, ki, :n],
                        start=(ki == 0), stop=(ki == DT - 1),
                    )
                nc.scalar.activation(
                    out=out_sb[:, do, ds(out_col0, n)], in_=ps, func=AF.Relu,
                    bias=bias_sb[:, do : do + 1], scale=1.0,
                )

        # ---- phase 1: q projection ----
        for sb in range(NSB):
            proj_T(wq_sb, bq_sb, q_in[sb], qTp, sb * 512, 512)

        # ---- phase 2: attention, software-pipelined ----
        residT = {}  # superblock -> tile [128, DT, 512]
        kTp = {}     # block -> k-projection tile
        W_sb = {}    # block -> masked scoresT (the banded weight matrix)
        av4 = {}     # superblock -> [128, DT, 512] aggregated v (4 blocks)
        sr4 = {}     # superblock -> [1, 512] colsums of W (4 blocks)

        def emit_kproj(b):
            k_t = k_tiles.pop(b)
            kp = ktp_p.tile([128, DT, 512], MM_DT, tag="kTp", name="kTp")
            proj_T(wk_sb, bk_sb, k_t, kp, 0, 512)
            kTp[b] = kp

        def emit_scores(b):
            # scT[k, w] = sum_d kTp[d, k] * qTp[d, w] for this block's keys
            sc_ps = ps_sc.tile([128, KC, 128], FP32, tag="sc_ps", name="sc_ps")
            for kc in range(KC):
                for ki in range(DT):
                    nc.tensor.matmul(
                        sc_ps[:, kc, :],
                        lhsT=kTp[b][:, ki, ts(kc, 128)],
                        rhs=qTp[:, ki, ts(b, 128)],
                        start=(ki == 0), stop=(ki == DT - 1),
                    )
            del kTp[b]
            # band mask -> sparse weight matrix W (bf16, zero off-band)
            w_t = w_p.tile([128, KC, 128], MM_DT, tag="W", name="W")
            nc.vector.tensor_tensor(w_t[:], sc_ps[:], maskT[:], op=OP.mult)
            W_sb[b] = w_t

        def emit_vagg(b):
            sb, col = b // 4, (b % 4) * 128
            v_t = v_tiles.pop(b)
            w_t = W_sb[b]
            av_ps = ps_av.tile([128, DT, 128], FP32, tag="av_ps", name="av_ps")
            for dc in range(DT):
                for kc in range(KC):
                    nc.tensor.matmul(
                        av_ps[:, dc, :],
                        lhsT=v_t[:, kc, ts(dc, 128)],
                        rhs=w_t[:, kc, :],
                        start=(kc == 0), stop=(kc == KC - 1),
                    )
            if use_vbias:
                # srow[w] = sum_k W[k, w]  (for the bias term)
                sr_ps = ps_misc.tile([1, 128], FP32, tag="sr_ps", name="sr_ps", bufs=1)
                for kc in range(KC):
                    nc.tensor.matmul(
                        sr_ps, lhsT=ones_colb, rhs=w_t[:, kc, :],
                        start=(kc == 0), stop=(kc == KC - 1),
                    )
            if col == 0:
                av4[sb] = av_p.tile([128, DT, 512], MM_DT, tag="av4", name="av4")
                if use_vbias:
                    sr4[sb] = small.tile([1, 512], MM_DT, tag="sr4", name="sr4", bufs=2)
            nc.scalar.activation(
                out=av4[sb][:, :, ds(col, 128)], in_=av_ps[:], func=AF.Copy, scale=1.0)
            if use_vbias:
                nc.scalar.activation(
                    out=sr4[sb][:, ds(col, 128)], in_=sr_ps, func=AF.Copy, scale=1.0)
            del W_sb[b]

        def stats_pair():
            """PSUM accumulators for the LN token sums + the squares tile."""
            S1 = ps_misc.tile([1, 512], FP32, tag="st", name="st_sum")
            S2 = ps_misc.tile([1, 512], FP32, tag="st", name="st_sq")
            sqt = hT_p.tile([128, DT, 512], MM_DT, tag="sq", name="sq")
            return S1, S2, sqt

        def emit_stats_dt(st, resid_t, dt):
            """LN stats for one d-tile; interleaved into the producer stream
            one chunk behind so the PE never waits on the DVE square."""
            S1, S2, sqt = st
            nc.vector.tensor_tensor(
                sqt[:, dt, :], resid_t[:, dt, :], resid_t[:, dt, :], op=OP.mult)
            nc.tensor.matmul(S1, lhsT=ones_colb, rhs=resid_t[:, dt, :],
                             start=(dt == 0), stop=(dt == DT - 1))
            nc.tensor.matmul(S2, lhsT=ones_colb, rhs=sqt[:, dt, :],
                             start=(dt == 0), stop=(dt == DT - 1))

        def emit_ln_finish(st, resid_t, g_sb, gb_sb, out_cb,
                           out_dt_chunked=None, affine=True):
            """Transposed LayerNorm over D given accumulated token sums.

            Scalar chain: varD = S2 + D*eps - D*mean^2; rstd = sqrt(D/varD).
            Broadcasts are rank-1 matmuls; apply is 2 DVE passes (+ ACT
            affine unless g==1, b==0).
            """
            S1, S2, _ = st
            mean = small.tile([1, 512], MM_DT, tag="mean", name="mean")
            nc.scalar.activation(out=mean, in_=S1, func=AF.Copy, scale=1.0 / D)
            m2d = small.tile([1, 512], FP32, tag="m2d", name="m2d")
            nc.vector.scalar_tensor_tensor(
                out=m2d, in0=mean, scalar=float(D), in1=mean,
                op0=OP.mult, op1=OP.mult,
            )
            varD = small.tile([1, 512], FP32, tag="varD", name="varD")
            nc.vector.scalar_tensor_tensor(
                out=varD, in0=S2, scalar=float(D) * EPS, in1=m2d,
                op0=OP.add, op1=OP.subtract,
            )
            r0 = small.tile([1, 512], FP32, tag="r0", name="r0")
            nc.vector.reciprocal_approx_fast(out=r0, in_=varD)
            rstd = small.tile([1, 512], FP16, tag="rstd", name="rstd")
            nc.scalar.activation(out=rstd, in_=r0, func=AF.Sqrt, scale=float(D))

            # bc tiles share the "st" tag/banks: S1/S2 are consumed by the
            # small-ops above before these are written.
            bcm = ps_misc.tile([128, 512], FP32, tag="st", name="bcm")
            nc.tensor.matmul(bcm, lhsT=ones_rowb, rhs=mean, start=True, stop=True)
            bcr = ps_misc.tile([128, 512], FP32, tag="st", name="bcr")
            nc.tensor.matmul(bcr, lhsT=ones_rowh, rhs=rstd, start=True, stop=True)

            # All subs first: they only need bcm, so they overlap the rstd
            # scalar chain; the mults drain once bcr lands.
            tmp = hT_p.tile([128, DT, 512], FP16, tag="tscr", name="tscr")
            for dt in range(DT):
                nc.vector.tensor_tensor(tmp[:, dt, :], resid_t[:, dt, :], bcm, op=OP.subtract)
            for dt in range(DT):
                if affine:
                    nc.vector.tensor_tensor(tmp[:, dt, :], tmp[:, dt, :], bcr, op=OP.mult)
                    nc.scalar.activation(
                        out=out_cb(dt), in_=tmp[:, dt, :], func=AF.Identity,
                        bias=gb_sb[:, dt : dt + 1], scale=g_sb[:, dt : dt + 1],
                    )
                else:
                    nc.vector.tensor_tensor(out_cb(dt), tmp[:, dt, :], bcr, op=OP.mult)
                if out_dt_chunked:
                    out_dt_chunked(dt)

        def emit_aoproj_sb(sb):
            # ao projection for a whole superblock at N=512 (the wv lhsT does
            # not depend on the block-diagonal attention structure), with the
            # residual add and LN1 stats pipelined into the do-chunk stream.
            residT[sb] = resid_p.tile([128, DT, 512], MM_DT, tag="residT", name="residT")
            st = stats_pair()
            for do in range(DT):
                ao_ps = ps_ao.tile([128, 512], FP32, tag="ao_ps", name="ao_ps")
                for ki in range(DT):
                    nc.tensor.matmul(
                        ao_ps, lhsT=late["wv"][:, ki, ts(do, 128)],
                        rhs=av4[sb][:, ki, :],
                        start=(ki == 0), stop=(ki == DT - 1) and not use_vbias,
                    )
                if use_vbias:
                    nc.tensor.matmul(
                        ao_ps, lhsT=bv_row[:, ts(do, 128)], rhs=sr4[sb],
                        start=False, stop=True,
                    )
                nc.vector.tensor_tensor(
                    residT[sb][:, do, :], ao_ps, qTp[:, do, ts(sb, 512)], op=OP.add,
                )
                if do >= 1:
                    emit_stats_dt(st, residT[sb], do - 1)
            emit_stats_dt(st, residT[sb], DT - 1)
            return st

        def emit_ln1_finish(sb, st):
            emit_ln_finish(st, residT[sb], late["g1"], late["gb1"],
                           lambda dt: xT[:, dt, ts(sb, 512)], affine=affine1)

        def emit_ffn1(sb):
            hT = hT_p.tile([128, DT, 512], MM_DT, tag="hT", name="hT")
            for ht in range(DT):
                ps = ps_proj.tile([128, 512], FP32, tag="proj_ps", name="ffn1_ps")
                for ki in range(DT):
                    nc.tensor.matmul(
                        ps, lhsT=late["w1"][:, ki, ts(ht, 128)], rhs=xT[:, ki, ts(sb, 512)],
                        start=(ki == 0), stop=(ki == DT - 1),
                    )
                nc.scalar.activation(
                    out=hT[:, ht, :], in_=ps, func=AF.Relu,
                    bias=late["b1"][:, ht : ht + 1], scale=1.0,
                )
            return hT

        def emit_ffn2(sb, hT):
            resid2 = resid_p.tile([128, DT, 512], MM_DT, tag="resid2", name="resid2")
            st = stats_pair()
            for dt in range(DT):
                ps = ps_proj.tile([128, 512], FP32, tag="proj_ps", name="ffn2_ps")
                for hi in range(DT):
                    nc.tensor.matmul(
                        ps, lhsT=late["w2"][:, hi, ts(dt, 128)], rhs=hT[:, hi, :],
                        start=(hi == 0), stop=(hi == DT - 1),
                    )
                nc.vector.scalar_tensor_tensor(
                    out=resid2[:, dt, :], in0=ps, scalar=late["b2"][:, dt : dt + 1],
                    in1=xT[:, dt, ts(sb, 512)], op0=OP.add, op1=OP.add,
                )
                if dt >= 1:
                    emit_stats_dt(st, resid2, dt - 1)
            emit_stats_dt(st, resid2, DT - 1)
            return resid2, st

        def emit_ln2_finish(sb, resid2, st):
            out_sb = out_p.tile([128, DT, 512], FP32, tag="out_sb", name="out_sb")

            def dma_dt(dt):
                nc.sync.dma_start(
                    out=outT_t[:, dt, ts(sb, 512)], in_=out_sb[:, dt, :]
                )

            emit_ln_finish(st, resid2, late["g2"], late["gb2"],
                           lambda dt: out_sb[:, dt, :], out_dt_chunked=dma_dt,
                           affine=affine2)

        # pipeline: k-proj(b), scoresT(b-1), v-agg(b-2); ao projection and
        # LN1 fire once per superblock when its 4 blocks' v-agg is emitted.
        for b in range(NBLK + 2):
            if b < NBLK:
                emit_kproj(b)
            if b == 0:
                prefetch_k(2)
                prefetch_v(0)
                prefetch_v(1)
                prefetch_v(2)
                load_late_consts()
            elif b < NBLK:
                prefetch_k(b + 2)
                prefetch_v(b + 2)
            if 1 <= b <= NBLK:
                emit_scores(b - 1)
            if 2 <= b <= NBLK + 1:
                emit_vagg(b - 2)
            if b - 2 == 3:  # v-agg(0..3) emitted -> superblock 0 ready
                ln1_st = emit_aoproj_sb(0)
            if b == 6:
                # LN1(0) broadcasts emit after kproj(6)/scores(5) so the PE
                # never waits on the rstd scalar chain.
                emit_ln1_finish(0, ln1_st)

        st1 = emit_aoproj_sb(1)
        hT0 = emit_ffn1(0)
        emit_ln1_finish(1, st1)
        r20, st20 = emit_ffn2(0, hT0)
        hT1 = emit_ffn1(1)
        emit_ln2_finish(0, r20, st20)
        r21, st21 = emit_ffn2(1, hT1)
        emit_ln2_finish(1, r21, st21)

    nc.finalize()
    return nc


def kernel(**inputs):
    # Specialize on actually-zero biases / identity LN affines (checked at
    # runtime; the general program is built when they are nontrivial).
    use_vbias = bool(np.any(np.asarray(inputs["b_v"], dtype=np.float32)))
    affine1 = not (
        np.all(np.asarray(inputs["ln1_g"], dtype=np.float32) == 1.0)
        and not np.any(np.asarray(inputs["ln1_b"], dtype=np.float32))
    )
    affine2 = not (
        np.all(np.asarray(inputs["ln2_g"], dtype=np.float32) == 1.0)
        and not np.any(np.asarray(inputs["ln2_b"], dtype=np.float32))
    )
    pkey = ("prog", use_vbias, affine1, affine2)
    if pkey not in _CACHE:
        _CACHE[pkey] = build_program(use_vbias, affine1, affine2)
    nc = _CACHE[pkey]

    import ml_dtypes

    f32 = lambda x: np.ascontiguousarray(np.asarray(x), dtype=np.float32)
    bf16 = lambda x: np.ascontiguousarray(np.asarray(x, dtype=np.float32).astype(ml_dtypes.bfloat16))
    query, key_, value = f32(inputs["query"]), f32(inputs["key"]), f32(inputs["value"])

    shared = {}
    packed = np.empty((128, 8 * DT), dtype=np.float32)
    for i, n in enumerate(("b_q", "b_k", "ffn_b1", "ffn_b2",
                           "ln1_g", "ln1_b", "ln2_g", "ln2_b")):
        packed[:, i * DT : (i + 1) * DT] = (
            np.asarray(inputs[n], dtype=np.float32).reshape(DT, 128).T)
    shared["constsP"] = packed
    for n in ("w_q", "w_k", "w_v", "ffn_w1", "ffn_w2"):
        shared[n] = bf16(inputs[n])
    shared["bv_row"] = bf16(np.asarray(inputs["b_v"], dtype=np.float32).reshape(1, D))
    # maskT[p, kc, w] = 1 where key (kc*128+p) belongs to window w of the block
    p_idx = np.arange(128)[:, None, None]
    kc_idx = np.arange(KC)[None, :, None]
    w_idx = np.arange(128)[None, None, :]
    shared["maskT"] = (w_idx == kc_idx * 32 + p_idx // 4).astype(np.float32)

    in_maps = []
    for c in range(NCORES):
        bi, half = c // 2, c % 2
        w0 = half * WPC
        m = dict(shared)
        m["qT"] = bf16(query[bi, w0 : w0 + WPC, :].T)
        m["kT"] = bf16(key_[bi, w0 * F : (w0 + WPC) * F, :].T)
        m["vN"] = bf16(value[bi, w0 * F : (w0 + WPC) * F, :])
        in_maps.append(m)

    res = run_bass_kernel_spmd(nc, in_maps, core_ids=list(range(NCORES)))
    _CACHE["last_result"] = res
    out = np.empty((B, SQ, D), dtype=np.float32)
    for c in range(NCORES):
        bi, half = c // 2, c % 2
        w0 = half * WPC
        out[bi, w0 : w0 + WPC, :] = res.results[c]["outT"].T
    return out


# revision 52
# speedup vs baseline: 1.1352x; 1.1352x over previous
"""Trainium2 Bass kernel for nn_AttentionSampling (sparse window attention block).

Sharding: 8 cores, data-parallel, 1024 windows (half a batch) per core; windows are
independent so there is no cross-core communication. q/k live in a transposed
[d, tokens] layout (host pre-transposes) so projections run weight-stationary;
v stays in natural [keys, d] layout so the banded attention aggregation can run
as PE matmuls against the masked score matrix.

Structure (per 128-window / 512-key block):
- k-proj (N=512 bf16 matmuls) -> scores computed directly TRANSPOSED
  ([keys, windows], 16 N=128 matmuls) -> DVE band-mask multiply produces the
  sparse weight matrix W [512 keys, 128 windows] in bf16.
- Attention output via aggregate-then-project: avT = v_nat.T-contracted with W
  (16 N=128 MMs per block); the wv projection + residual add + LN1 stats then
  run once per 512-token superblock at N=512 (wv does not see the block
  structure). 4x fewer v-projection MACs than project-then-aggregate, no PE
  transposes.
- LayerNorm stats are bf16 ones-matmuls pipelined into the producer streams;
  mean/rstd broadcasts are rank-1 matmuls; the scalar chain is 5 hops; the
  apply is 2 DVE passes (ACT affine only when g != 1 or b != 0, decided at
  runtime from the actual inputs, as are the zero v/ffn biases).
- Software pipeline: iteration b emits k-proj(b), scoresT(b-1), v-agg(b-2);
  every PE group's ACT/DVE dependencies are >=1 block old.
- DMA issue follows global need order across the sync/scalar/gpsimd queues
  (aggregate ~330 GB/s is shared; engine boot blocks issue until ~7.6us);
  PE warmup matmuls bridge the initial fill so the HAM clock gate is open
  when real work starts.
- LN2 apply + output DMA chunked per d-tile to shrink the serial tail.
"""

import sys
import types

# If BASS_TRACE is set in an environment whose antenv package lacks
# axon_hooks, run_bass_kernel_spmd would crash on import; provide a stub
# (a None hook makes bass_utils skip tracing gracefully).
try:
    import antenv.axon_hooks  # noqa: F401
except ImportError:
    _m = types.ModuleType("antenv.axon_hooks")
    _m.get_axon_ntff_profile_hook = lambda: None
    _m.set_axon_ntff_profile_hook = lambda h: None
    sys.modules["antenv.axon_hooks"] = _m
    try:
        import antenv

        antenv.axon_hooks = _m
    except ImportError:
        pass

import contextlib

import numpy as np

import concourse.bass as bass
import concourse.bacc as bacc_mod
import concourse.mybir as mybir
import concourse.tile as tile
from concourse.bass import ts, ds
from concourse.bass_utils import run_bass_kernel_spmd

FP32 = mybir.dt.float32
FP16 = mybir.dt.float16
AF = mybir.ActivationFunctionType
OP = mybir.AluOpType

MM_DT = mybir.dt.bfloat16  # matmul operands
# The residual stream and LN stats run in bf16 (fp16 matmuls measured at HALF
# the bf16 rate on TRN2 HW despite the cost model saying otherwise; bf16
# element quantization only costs ~1e-2 worst-element relative error vs the
# 2e-2 gate). PSUM accumulation is fp32. rstd and the centered apply scratch
# stay fp16 (11-bit mantissa) since they multiply the output directly.

B, SQ, SK, D, F = 4, 2048, 8192, 512, 4
NCORES = 8
WPC = B * SQ // NCORES        # 1024 windows (= tokens) per core
KPC = WPC * F                 # 4096 keys per core
NBLK = WPC // 128             # 8 attention blocks: 128 windows / 512 keys
NSB = WPC // 512              # 2 superblocks of 512 tokens
DT = D // 128                 # 4 d-tiles
KC = 4                        # key chunks per block (512 keys / 128)
EPS = 1e-5
N_WARMUP = 10                 # PE warmup matmuls during initial DMA fill

_CACHE = {}


def build_program(use_vbias=True, affine1=True, affine2=True):
    nc = bacc_mod.Bacc(None, target_bir_lowering=False)

    qT_d = nc.dram_tensor("qT", [D, WPC], MM_DT, kind="ExternalInput")
    kT_d = nc.dram_tensor("kT", [D, KPC], MM_DT, kind="ExternalInput")
    vN_d = nc.dram_tensor("vN", [KPC, D], MM_DT, kind="ExternalInput")
    wq_d = nc.dram_tensor("w_q", [D, D], MM_DT, kind="ExternalInput")
    wk_d = nc.dram_tensor("w_k", [D, D], MM_DT, kind="ExternalInput")
    wv_d = nc.dram_tensor("w_v", [D, D], MM_DT, kind="ExternalInput")
    w1_d = nc.dram_tensor("ffn_w1", [D, D], MM_DT, kind="ExternalInput")
    w2_d = nc.dram_tensor("ffn_w2", [D, D], MM_DT, kind="ExternalInput")
    # All [D] bias/gain vectors are packed host-side into one [128, 8*DT]
    # tensor (order: b_q, b_k, ffn_b1, ffn_b2, ln1_g, ln1_b, ln2_g, ln2_b):
    # loading them individually as 4-byte-element gather DMAs costs 4-9us of
    # descriptor generation EACH and clogs the issuing engine's queue.
    consts_d = nc.dram_tensor("constsP", [128, 8 * DT], FP32, kind="ExternalInput")
    bvr_d = nc.dram_tensor("bv_row", [1, D], MM_DT, kind="ExternalInput")
    maskT_d = nc.dram_tensor("maskT", [128, KC, 128], FP32, kind="ExternalInput")
    outT_d = nc.dram_tensor("outT", [D, WPC], FP32, kind="ExternalOutput")

    qT_t = qT_d.rearrange("(o p) n -> p o n", p=128)
    kT_t = kT_d.rearrange("(o p) n -> p o n", p=128)
    vN_t = vN_d.rearrange("(nb kc p) d -> p nb kc d", p=128, kc=KC)
    outT_t = outT_d.rearrange("(o p) n -> p o n", p=128)

    with tile.TileContext(nc) as tc, contextlib.ExitStack() as ctx:
        # PSUM budget is 8 banks x 2KB: proj(2) + sc(1) + av(1) + ao(1|2) +
        # stats/bc shared tag(2) [+ srow(1) on the biased path] = 8.
        singles = ctx.enter_context(tc.tile_pool(name="singles", bufs=1))
        qin_p = ctx.enter_context(tc.tile_pool(name="qin", bufs=2))
        kin_p = ctx.enter_context(tc.tile_pool(name="kin", bufs=3))
        vin_p = ctx.enter_context(tc.tile_pool(name="vin", bufs=5))
        ktp_p = ctx.enter_context(tc.tile_pool(name="ktp", bufs=2))
        w_p = ctx.enter_context(tc.tile_pool(name="wsb", bufs=2))
        av_p = ctx.enter_context(tc.tile_pool(name="avsb", bufs=2))
        resid_p = ctx.enter_context(tc.tile_pool(name="resid", bufs=2))
        hT_p = ctx.enter_context(tc.tile_pool(name="hT", bufs=2))
        out_p = ctx.enter_context(tc.tile_pool(name="outp", bufs=2))
        small = ctx.enter_context(tc.tile_pool(name="small", bufs=1))
        ps_proj = ctx.enter_context(tc.tile_pool(name="ps_proj", bufs=2, space="PSUM"))
        ps_sc = ctx.enter_context(tc.tile_pool(name="ps_sc", bufs=1, space="PSUM"))
        ps_av = ctx.enter_context(tc.tile_pool(name="ps_av", bufs=1, space="PSUM"))
        ps_ao = ctx.enter_context(
            tc.tile_pool(name="ps_ao", bufs=1 if use_vbias else 2, space="PSUM"))
        ps_misc = ctx.enter_context(tc.tile_pool(name="ps_misc", bufs=2, space="PSUM"))

        # DMAs are spread across engine queues: single-queue DMA bandwidth
        # (~170-200 GB/s) is well below the core's aggregate, and the early
        # input fill is latency-critical for the PE.
        def load_w(d, tg, eng=None):
            t = singles.tile([128, DT, 512], MM_DT, tag=tg)
            (eng or nc.scalar).dma_start(out=t, in_=d.rearrange("(o p) n -> p o n", p=128))
            return t

        consts_sb = singles.tile([128, 8 * DT], FP32, tag="constsP")
        _CONST_IDX = {"b_q": 0, "b_k": 1, "ffn_b1": 2, "ffn_b2": 3,
                      "ln1_g": 4, "ln1_b": 5, "ln2_g": 6, "ln2_b": 7}

        def load_b(name):
            i = _CONST_IDX[name]
            return consts_sb[:, i * DT : (i + 1) * DT]

        # Warmup scratch needs no DMA: memset, then matmuls on it below.
        # Total DMA bandwidth (~330 GB/s) is shared across queues, so issue
        # order must be GLOBAL need order; multiple queues only parallelize
        # the ~0.7-1.9us per-issue instruction cost. Engine boot blocks all
        # DMA issue until ~7.6us, so the first-needed bytes are minimal:
        # wq+q0 (warmup covers them), then q1, wk+k0, k1; v/wv/ffn weights
        # stream in behind.
        warm_sb = singles.tile([128, 512], MM_DT, tag="warm")
        nc.gpsimd.memset(warm_sb, 0.001)
        ones_colb = singles.tile([128, 1], MM_DT, tag="ones_colb")
        nc.gpsimd.memset(ones_colb, 1.0)
        ones_rowb = singles.tile([1, 128], MM_DT, tag="ones_rowb")
        nc.gpsimd.memset(ones_rowb, 1.0)
        ones_rowh = singles.tile([1, 128], FP16, tag="ones_rowh")
        nc.gpsimd.memset(ones_rowh, 1.0)

        k_tiles = {}
        v_tiles = {}

        def prefetch_k(b):
            if b >= NBLK:
                return
            k_t = kin_p.tile([128, DT, 512], MM_DT, tag="k_in", name="k_in")
            nc.sync.dma_start(out=k_t, in_=kT_t[:, :, ts(b, 512)])
            k_tiles[b] = k_t

        def prefetch_v(b):
            if b >= NBLK:
                return
            v_t = vin_p.tile([128, KC, 512], MM_DT, tag="v_in", name="v_in")
            nc.gpsimd.dma_start(out=v_t, in_=vN_t[:, b, :, :])
            v_tiles[b] = v_t

        wq_sb = load_w(wq_d, "wq")
        nc.scalar.dma_start(out=consts_sb, in_=consts_d[:, :])
        q_in = []
        for sb in range(NSB):
            t = qin_p.tile([128, DT, 512], MM_DT, tag="q_in", name="q_in")
            q_in.append(t)
        nc.sync.dma_start(out=q_in[0], in_=qT_t[:, :, ts(0, 512)])
        wk_sb = load_w(wk_d, "wk")
        nc.sync.dma_start(out=q_in[1], in_=qT_t[:, :, ts(1, 512)])
        prefetch_k(0)
        bq_sb = load_b("b_q")
        bk_sb = load_b("b_k")
        prefetch_k(1)
        maskT = singles.tile([128, KC, 128], FP32, tag="maskT")
        nc.gpsimd.dma_start(out=maskT, in_=maskT_d[:, :, :])
        bv_row = None
        if use_vbias:
            bv_row = singles.tile([1, 512], MM_DT, tag="bv_row")
            nc.scalar.dma_start(out=bv_row, in_=bvr_d[:, :])

        late = {}

        def load_late_consts():
            late["wv"] = load_w(wv_d, "wv", eng=nc.gpsimd)
            late["g1"] = load_b("ln1_g")
            late["gb1"] = load_b("ln1_b")
            late["w1"] = load_w(w1_d, "w1", eng=nc.gpsimd)
            late["b1"] = load_b("ffn_b1")
            late["w2"] = load_w(w2_d, "w2", eng=nc.gpsimd)
            late["b2"] = load_b("ffn_b2")
            late["g2"] = load_b("ln2_g")
            late["gb2"] = load_b("ln2_b")

        # ---- PE warmup: trip the HAM clock gate while DMAs fill ----
        for i in range(N_WARMUP):
            wps = ps_proj.tile([128, 512], FP32, tag="proj_ps", name="warm_ps")
            nc.tensor.matmul(wps, lhsT=warm_sb[:, :128], rhs=warm_sb,
                             start=True, stop=True)

        qTp = singles.tile([128, DT, WPC], MM_DT, tag="qTp")
        xT = singles.tile([128, DT, WPC], MM_DT, tag="xT")

        def proj_T(w_sb, bias_sb, in_sb, out_sb, out_col0, n):
            for do in range(DT):
                ps = ps_proj.tile([128, 512], FP32, tag="proj_ps", name="proj_ps")
                ps = ps[:, :n]
                for ki in range(DT):
                    nc.tensor.matmul(
                        ps, lhsT=w_sb[:, ki, ts(do, 128)], rhs=in_sb[:, ki, :n],
                        start=(ki == 0), stop=(ki == DT - 1),
                    )
                nc.scalar.activation(
                    out=out_sb[:, do, ds(out_col0, n)], in_=ps, func=AF.Relu,
                    bias=bias_sb[:, do : do + 1], scale=1.0,
                )

        # ---- phase 1: q projection ----
        for sb in range(NSB):
            proj_T(wq_sb, bq_sb, q_in[sb], qTp, sb * 512, 512)

        # ---- phase 2: attention, software-pipelined ----
        residT = {}  # superblock -> tile [128, DT, 512]
        kTp = {}     # block -> k-projection tile
        W_sb = {}    # block -> masked scoresT (the banded weight matrix)
        av4 = {}     # superblock -> [128, DT, 512] aggregated v (4 blocks)
        sr4 = {}     # superblock -> [1, 512] colsums of W (4 blocks)

        def emit_kproj(b):
            k_t = k_tiles.pop(b)
            kp = ktp_p.tile([128, DT, 512], MM_DT, tag="kTp", name="kTp")
            proj_T(wk_sb, bk_sb, k_t, kp, 0, 512)
            kTp[b] = kp

        def emit_scores(b):
            # scT[k, w] = sum_d kTp[d, k] * qTp[d, w] for this block's keys
            sc_ps = ps_sc.tile([128, KC, 128], FP32, tag="sc_ps", name="sc_ps")
            for kc in range(KC):
                for ki in range(DT):
                    nc.tensor.matmul(
                        sc_ps[:, kc, :],
                        lhsT=kTp[b][:, ki, ts(kc, 128)],
                        rhs=qTp[:, ki, ts(b, 128)],
                        start=(ki == 0), stop=(ki == DT - 1),
                    )
            del kTp[b]
            # band mask -> sparse weight matrix W (bf16, zero off-band)
            w_t = w_p.tile([128, KC, 128], MM_DT, tag="W", name="W")
            nc.vector.tensor_tensor(w_t[:], sc_ps[:], maskT[:], op=OP.mult)
            W_sb[b] = w_t

        def emit_vagg(b):
            sb, col = b // 4, (b % 4) * 128
            v_t = v_tiles.pop(b)
            w_t = W_sb[b]
            av_ps = ps_av.tile([128, DT, 128], FP32, tag="av_ps", name="av_ps")
            for dc in range(DT):
                for kc in range(KC):
                    nc.tensor.matmul(
                        av_ps[:, dc, :],
                        lhsT=v_t[:, kc, ts(dc, 128)],
                        rhs=w_t[:, kc, :],
                        start=(kc == 0), stop=(kc == KC - 1),
                    )
            if use_vbias:
                # srow[w] = sum_k W[k, w]  (for the bias term)
                sr_ps = ps_misc.tile([1, 128], FP32, tag="sr_ps", name="sr_ps", bufs=1)
                for kc in range(KC):
                    nc.tensor.matmul(
                        sr_ps, lhsT=ones_colb, rhs=w_t[:, kc, :],
                        start=(kc == 0), stop=(kc == KC - 1),
                    )
            if col == 0:
                av4[sb] = av_p.tile([128, DT, 512], MM_DT, tag="av4", name="av4")
                if use_vbias:
                    sr4[sb] = small.tile([1, 512], MM_DT, tag="sr4", name="sr4", bufs=2)
            nc.scalar.activation(
                out=av4[sb][:, :, ds(col, 128)], in_=av_ps[:], func=AF.Copy, scale=1.0)
            if use_vbias:
                nc.scalar.activation(
                    out=sr4[sb][:, ds(col, 128)], in_=sr_ps, func=AF.Copy, scale=1.0)
            del W_sb[b]

        def stats_pair():
            """PSUM accumulators for the LN token sums + the squares tile."""
            S1 = ps_misc.tile([1, 512], FP32, tag="st", name="st_sum")
            S2 = ps_misc.tile([1, 512], FP32, tag="st", name="st_sq")
            sqt = hT_p.tile([128, DT, 512], MM_DT, tag="sq", name="sq")
            return S1, S2, sqt

        def emit_stats_dt(st, resid_t, dt):
            """LN stats for one d-tile; interleaved into the producer stream
            one chunk behind so the PE never waits on the square. The square
            runs on GPSIMD (idle in these windows, SBUF-only op) so the
            in-order DVE queue can't head-of-line block the stats."""
            S1, S2, sqt = st
            nc.gpsimd.tensor_tensor(
                sqt[:, dt, :], resid_t[:, dt, :], resid_t[:, dt, :], op=OP.mult)
            nc.tensor.matmul(S1, lhsT=ones_colb, rhs=resid_t[:, dt, :],
                             start=(dt == 0), stop=(dt == DT - 1))
            nc.tensor.matmul(S2, lhsT=ones_colb, rhs=sqt[:, dt, :],
                             start=(dt == 0), stop=(dt == DT - 1))

        def emit_ln_finish(st, resid_t, g_sb, gb_sb, out_cb,
                           out_dt_chunked=None, affine=True):
            """Transposed LayerNorm over D given accumulated token sums.

            Scalar chain: varD = S2 + D*eps - D*mean^2; rstd = sqrt(D/varD).
            Broadcasts are rank-1 matmuls; apply is 2 DVE passes (+ ACT
            affine unless g==1, b==0).
            """
            S1, S2, _ = st
            mean = small.tile([1, 512], MM_DT, tag="mean", name="mean")
            nc.scalar.activation(out=mean, in_=S1, func=AF.Copy, scale=1.0 / D)
            m2d = small.tile([1, 512], FP32, tag="m2d", name="m2d")
            nc.vector.scalar_tensor_tensor(
                out=m2d, in0=mean, scalar=float(D), in1=mean,
                op0=OP.mult, op1=OP.mult,
            )
            varD = small.tile([1, 512], FP32, tag="varD", name="varD")
            nc.vector.scalar_tensor_tensor(
                out=varD, in0=S2, scalar=float(D) * EPS, in1=m2d,
                op0=OP.add, op1=OP.subtract,
            )
            r0 = small.tile([1, 512], FP32, tag="r0", name="r0")
            nc.vector.reciprocal_approx_fast(out=r0, in_=varD)
            rstd = small.tile([1, 512], FP16, tag="rstd", name="rstd")
            nc.scalar.activation(out=rstd, in_=r0, func=AF.Sqrt, scale=float(D))

            # bc tiles share the "st" tag/banks: S1/S2 are consumed by the
            # small-ops above before these are written.
            bcm = ps_misc.tile([128, 512], FP32, tag="st", name="bcm")
            nc.tensor.matmul(bcm, lhsT=ones_rowb, rhs=mean, start=True, stop=True)
            bcr = ps_misc.tile([128, 512], FP32, tag="st", name="bcr")
            nc.tensor.matmul(bcr, lhsT=ones_rowh, rhs=rstd, start=True, stop=True)

            # All subs first: they only need bcm, so they overlap the rstd
            # scalar chain; the mults drain once bcr lands.
            tmp = hT_p.tile([128, DT, 512], FP16, tag="tscr", name="tscr")
            for dt in range(DT):
                nc.vector.tensor_tensor(tmp[:, dt, :], resid_t[:, dt, :], bcm, op=OP.subtract)
            for dt in range(DT):
                if affine:
                    nc.vector.tensor_tensor(tmp[:, dt, :], tmp[:, dt, :], bcr, op=OP.mult)
                    nc.scalar.activation(
                        out=out_cb(dt), in_=tmp[:, dt, :], func=AF.Identity,
                        bias=gb_sb[:, dt : dt + 1], scale=g_sb[:, dt : dt + 1],
                    )
                else:
                    nc.vector.tensor_tensor(out_cb(dt), tmp[:, dt, :], bcr, op=OP.mult)
                if out_dt_chunked:
                    out_dt_chunked(dt)

        def emit_aoproj_sb(sb):
            # ao projection for a whole superblock at N=512 (the wv lhsT does
            # not depend on the block-diagonal attention structure), with the
            # residual add and LN1 stats pipelined into the do-chunk stream.
            residT[sb] = resid_p.tile([128, DT, 512], MM_DT, tag="residT", name="residT")
            st = stats_pair()
            for do in range(DT):
                ao_ps = ps_ao.tile([128, 512], FP32, tag="ao_ps", name="ao_ps")
                for ki in range(DT):
                    nc.tensor.matmul(
                        ao_ps, lhsT=late["wv"][:, ki, ts(do, 128)],
                        rhs=av4[sb][:, ki, :],
                        start=(ki == 0), stop=(ki == DT - 1) and not use_vbias,
                    )
                if use_vbias:
                    nc.tensor.matmul(
                        ao_ps, lhsT=bv_row[:, ts(do, 128)], rhs=sr4[sb],
                        start=False, stop=True,
                    )
                nc.vector.tensor_tensor(
                    residT[sb][:, do, :], ao_ps, qTp[:, do, ts(sb, 512)], op=OP.add,
                )
                if do >= 1:
                    emit_stats_dt(st, residT[sb], do - 1)
            emit_stats_dt(st, residT[sb], DT - 1)
            return st

        def emit_ln1_finish(sb, st):
            emit_ln_finish(st, residT[sb], late["g1"], late["gb1"],
                           lambda dt: xT[:, dt, ts(sb, 512)], affine=affine1)

        def emit_ffn1(sb):
            hT = hT_p.tile([128, DT, 512], MM_DT, tag="hT", name="hT")
            for ht in range(DT):
                ps = ps_proj.tile([128, 512], FP32, tag="proj_ps", name="ffn1_ps")
                for ki in range(DT):
                    nc.tensor.matmul(
                        ps, lhsT=late["w1"][:, ki, ts(ht, 128)], rhs=xT[:, ki, ts(sb, 512)],
                        start=(ki == 0), stop=(ki == DT - 1),
                    )
                nc.scalar.activation(
                    out=hT[:, ht, :], in_=ps, func=AF.Relu,
                    bias=late["b1"][:, ht : ht + 1], scale=1.0,
                )
            return hT

        def emit_ffn2(sb, hT):
            resid2 = resid_p.tile([128, DT, 512], MM_DT, tag="resid2", name="resid2")
            st = stats_pair()
            for dt in range(DT):
                ps = ps_proj.tile([128, 512], FP32, tag="proj_ps", name="ffn2_ps")
                for hi in range(DT):
                    nc.tensor.matmul(
                        ps, lhsT=late["w2"][:, hi, ts(dt, 128)], rhs=hT[:, hi, :],
                        start=(hi == 0), stop=(hi == DT - 1),
                    )
                nc.vector.scalar_tensor_tensor(
                    out=resid2[:, dt, :], in0=ps, scalar=late["b2"][:, dt : dt + 1],
                    in1=xT[:, dt, ts(sb, 512)], op0=OP.add, op1=OP.add,
                )
                if dt >= 1:
                    emit_stats_dt(st, resid2, dt - 1)
            emit_stats_dt(st, resid2, DT - 1)
            return resid2, st

        def emit_ln2_finish(sb, resid2, st):
            out_sb = out_p.tile([128, DT, 512], FP32, tag="out_sb", name="out_sb")

            def dma_dt(dt):
                nc.sync.dma_start(
                    out=outT_t[:, dt, ts(sb, 512)], in_=out_sb[:, dt, :]
                )

            emit_ln_finish(st, resid2, late["g2"], late["gb2"],
                           lambda dt: out_sb[:, dt, :], out_dt_chunked=dma_dt,
                           affine=affine2)

        # pipeline: k-proj(b), scoresT(b-1), v-agg(b-2); ao projection and
        # LN1 fire once per superblock when its 4 blocks' v-agg is emitted.
        for b in range(NBLK + 2):
            if b < NBLK:
                emit_kproj(b)
            if b == 0:
                prefetch_k(2)
                prefetch_v(0)
                prefetch_v(1)
                prefetch_v(2)
                load_late_consts()
            elif b < NBLK:
                prefetch_k(b + 2)
                prefetch_v(b + 2)
            if 1 <= b <= NBLK:
                emit_scores(b - 1)
            if 2 <= b <= NBLK + 1:
                emit_vagg(b - 2)
            if b - 2 == 3:  # v-agg(0..3) emitted -> superblock 0 ready
                ln1_st = emit_aoproj_sb(0)
            if b == 6:
                # LN1(0) broadcasts emit after kproj(6)/scores(5) so the PE
                # never waits on the rstd scalar chain.
                emit_ln1_finish(0, ln1_st)

        st1 = emit_aoproj_sb(1)
        hT0 = emit_ffn1(0)
        emit_ln1_finish(1, st1)
        r20, st20 = emit_ffn2(0, hT0)
        hT1 = emit_ffn1(1)
        emit_ln2_finish(0, r20, st20)
        r21, st21 = emit_ffn2(1, hT1)
        emit_ln2_finish(1, r21, st21)

    nc.finalize()
    return nc


def kernel(**inputs):
    # Specialize on actually-zero biases / identity LN affines (checked at
    # runtime; the general program is built when they are nontrivial).
    use_vbias = bool(np.any(np.asarray(inputs["b_v"], dtype=np.float32)))
    affine1 = not (
        np.all(np.asarray(inputs["ln1_g"], dtype=np.float32) == 1.0)
        and not np.any(np.asarray(inputs["ln1_b"], dtype=np.float32))
    )
    affine2 = not (
        np.all(np.asarray(inputs["ln2_g"], dtype=np.float32) == 1.0)
        and not np.any(np.asarray(inputs["ln2_b"], dtype=np.float32))
    )
    pkey = ("prog", use_vbias, affine1, affine2)
    if pkey not in _CACHE:
        _CACHE[pkey] = build_program(use_vbias, affine1, affine2)
    nc = _CACHE[pkey]

    import ml_dtypes

    f32 = lambda x: np.ascontiguousarray(np.asarray(x), dtype=np.float32)
    bf16 = lambda x: np.ascontiguousarray(np.asarray(x, dtype=np.float32).astype(ml_dtypes.bfloat16))
    query, key_, value = f32(inputs["query"]), f32(inputs["key"]), f32(inputs["value"])

    shared = {}
    packed = np.empty((128, 8 * DT), dtype=np.float32)
    for i, n in enumerate(("b_q", "b_k", "ffn_b1", "ffn_b2",
                           "ln1_g", "ln1_b", "ln2_g", "ln2_b")):
        packed[:, i * DT : (i + 1) * DT] = (
            np.asarray(inputs[n], dtype=np.float32).reshape(DT, 128).T)
    shared["constsP"] = packed
    for n in ("w_q", "w_k", "w_v", "ffn_w1", "ffn_w2"):
        shared[n] = bf16(inputs[n])
    shared["bv_row"] = bf16(np.asarray(inputs["b_v"], dtype=np.float32).reshape(1, D))
    # maskT[p, kc, w] = 1 where key (kc*128+p) belongs to window w of the block
    p_idx = np.arange(128)[:, None, None]
    kc_idx = np.arange(KC)[None, :, None]
    w_idx = np.arange(128)[None, None, :]
    shared["maskT"] = (w_idx == kc_idx * 32 + p_idx // 4).astype(np.float32)

    in_maps = []
    for c in range(NCORES):
        bi, half = c // 2, c % 2
        w0 = half * WPC
        m = dict(shared)
        m["qT"] = bf16(query[bi, w0 : w0 + WPC, :].T)
        m["kT"] = bf16(key_[bi, w0 * F : (w0 + WPC) * F, :].T)
        m["vN"] = bf16(value[bi, w0 * F : (w0 + WPC) * F, :])
        in_maps.append(m)

    res = run_bass_kernel_spmd(nc, in_maps, core_ids=list(range(NCORES)))
    _CACHE["last_result"] = res
    out = np.empty((B, SQ, D), dtype=np.float32)
    for c in range(NCORES):
        bi, half = c // 2, c % 2
        w0 = half * WPC
        out[bi, w0 : w0 + WPC, :] = res.results[c]["outT"].T
    return out
